# revision 2
# baseline (speedup 1.0000x reference)
"""Causal multi-head self-attention with RoPE on 8 Trainium2 NeuronCores.

Problem shapes (hardcoded): x [2, 2048, 1024], wq/wk/wv/wo [1024, 1024],
16 heads, head dim 64, rope theta 1000.0.

Sharding: tensor-parallel over heads -- each of the 8 cores owns 2 heads
(128 of the 1024 hidden dims). wq/wk/wv are column-sharded (rows of the
[out, in] weights), wo is row-sharded; the all-reduce of the 8 partial
outputs is done on the host during the gather/unshard step.

Design notes (v2 -- tuned for warm-PE occupancy):
  - PE cost on TRN2 is 1 moving-column/cycle with a HAM clock gate
    (2.4 GHz warm / 1.2 GHz cold), so the schedule aims for a dense,
    uninterrupted matmul stream: all projections (both batches) first,
    then attention, with wo-projections deferred one query-tile and
    interleaved into the attention matmul stream.
  - RoPE pairs are permuted to (row r, row r+32) within each 64-row head
    block; rot = psb*C + swap32(psb*S') where the partition swap runs as
    4 small SBUF->SBUF DMAs off the critical engines.
  - Scores are computed as S^T tiles [128 keys, 512 queries]; two key
    blocks share one 2-bank PSUM slot so exp() runs as [128, 1024] calls
    (scalar-engine activation overhead is 352 cycles/call).
  - Softmax needs no max-subtraction (scores are O(1)); the denominator
    comes from a ones-column appended to V; normalization uses a gpsimd
    partition_broadcast of the reciprocal row.
  - All matmul inputs bf16; fp32 PSUM accumulation; bf16 partial outputs
    (summed in fp32 on host).
"""

import sys

sys.path.insert(0, "/opt/trn_rl_repo")

from collections import deque

import ml_dtypes
import numpy as np

import concourse.bacc as bacc
import concourse.tile as tile
from concourse import mybir

F32 = mybir.dt.float32
BF16 = mybir.dt.bfloat16
EXP = mybir.ActivationFunctionType.Exp

B = 2
T = 2048
D = 1024
H = 16
DK = 64
NCORES = 8
HPC = H // NCORES      # heads per core = 2
E = HPC * DK           # local out dims per core = 128
DC = D // 128          # 8 chunks of the contraction dim
QT = T // 512          # 4 query tiles of 512
TT = T // 128          # 16 key/value tiles of 128
ROPE_THETA = 1000.0


def build_nc():
    nc = bacc.Bacc("TRN2", target_bir_lowering=False, debug=False,
                   num_devices=NCORES)

    xT = nc.dram_tensor("xT", [B, DC, 128, T], BF16, kind="ExternalInput")
    wqT = nc.dram_tensor("wqT", [128, DC, E], BF16, kind="ExternalInput")
    wkT = nc.dram_tensor("wkT", [128, DC, E], BF16, kind="ExternalInput")
    wvT = nc.dram_tensor("wvT", [128, DC, E], BF16, kind="ExternalInput")
    woT = nc.dram_tensor("woT", [128, DC, 128], BF16, kind="ExternalInput")
    ctab = nc.dram_tensor("ctab", [128, T], BF16, kind="ExternalInput")
    stab = nc.dram_tensor("stab", [128, T], BF16, kind="ExternalInput")
    tri = nc.dram_tensor("tri", [128, 128], BF16, kind="ExternalInput")
    eye = nc.dram_tensor("eye", [128, 128], BF16, kind="ExternalInput")
    outT = nc.dram_tensor("outT", [B, QT, 128, DC, 512], BF16,
                          kind="ExternalOutput")

    from contextlib import ExitStack

    with tile.TileContext(nc) as tc, ExitStack() as est:
        pool = lambda name, bufs, **kw: est.enter_context(
            tc.tile_pool(name=name, bufs=bufs, **kw))
        constp = pool("const", 1)
        xtp = pool("xt", 2 * DC)
        rotp = pool("rot", 4)
        psbp = pool("psb", 2)
        m1p = pool("m1", 2)
        m2p = pool("m2", 2)
        m2sp = pool("m2s", 2)
        vtsp = pool("vts", 2)
        vtp = pool("vt", 2)
        ptp = pool("pt", 3)
        catp = pool("cat", 3)
        osbp = pool("osb", 2)
        recp = pool("rec", 2)
        psB = pool("psB", 2, space="PSUM")   # proj accum + S^T pair slots
        psH = pool("psH", 2, space="PSUM")   # head accum [65, 512]
        psO = pool("psO", 2, space="PSUM")   # wo out + V transposes

        # ---- constant tiles ----
        wq_sb = constp.tile([128, DC, E], BF16, tag="wq")
        wk_sb = constp.tile([128, DC, E], BF16, tag="wk")
        wv_sb = constp.tile([128, DC, E], BF16, tag="wv")
        wo_sb = constp.tile([128, DC, 128], BF16, tag="wo")
        ct_sb = constp.tile([128, T], BF16, tag="ct")
        st_sb = constp.tile([128, T], BF16, tag="st")
        tri_sb = constp.tile([128, 128], BF16, tag="tri")
        eye_sb = constp.tile([128, 128], BF16, tag="eye")

        # Warm the scalar-engine exp tables while the input DMAs run.
        dum = constp.tile([128, 16], F32, tag="dum")
        nc.vector.memset(dum[:], 0.0)
        nc.scalar.activation(dum[:], dum[:], EXP)

        def dma_in(dst, src, nchunks):
            step = 128 // nchunks
            for i in range(nchunks):
                nc.sync.dma_start(dst[i * step:(i + 1) * step],
                                  src[i * step:(i + 1) * step])

        xts = {}
        for b in range(B):
            for dc in range(DC):
                xts[(b, dc)] = xtp.tile([128, T], BF16, tag="xt",
                                        name=f"xt_{b}_{dc}")

        # Input DMA issue order = arrival priority.
        dma_in(wq_sb, wqT, 4)
        dma_in(ct_sb, ctab, 4)
        dma_in(st_sb, stab, 4)
        dma_in(wk_sb, wkT, 4)
        dma_in(wv_sb, wvT, 4)
        for dc in range(DC):
            dma_in(xts[(0, dc)], xT[0, dc], 4)
        dma_in(wo_sb, woT, 4)
        dma_in(tri_sb, tri, 1)
        dma_in(eye_sb, eye, 1)
        for dc in range(DC):
            dma_in(xts[(1, dc)], xT[1, dc], 4)

        rots = {}   # (b, 'q'|'k') -> [128, T] bf16
        vts_all = {}  # b -> vt tile [128, TT, HPC, 65]

        def rope_chain(b, slot, rot, half):
            co = half * 1024
            psb = psbp.tile([128, 1024], BF16, tag="psb")
            nc.scalar.copy(psb[:], slot[:])
            m1 = m1p.tile([128, 1024], BF16, tag="m1")
            nc.vector.tensor_mul(m1[:], psb[:], ct_sb[:, co:co + 1024])
            m2 = m2p.tile([128, 1024], BF16, tag="m2")
            nc.gpsimd.tensor_mul(m2[:], psb[:], st_sb[:, co:co + 1024])
            m2s = m2sp.tile([128, 1024], BF16, tag="m2s")
            for blk in range(4):
                s = (blk ^ 1) * 32
                nc.sync.dma_start(m2s[blk * 32:blk * 32 + 32, :],
                                  m2[s:s + 32, :])
            nc.gpsimd.tensor_add(rot[:, co:co + 1024], m1[:], m2s[:])

        def v_chain(b, slot, half):
            vt = vts_all[b]
            vts = vtsp.tile([128, 1024], BF16, tag="vts")
            nc.vector.tensor_copy(vts[:], slot[:])
            for j4 in range(8):
                tch = half * 8 + j4
                pst = psO.tile([128, 128], BF16, tag="psO",
                               name=f"pst_{b}_{tch}")
                nc.tensor.transpose(pst[:], vts[:, j4 * 128:(j4 + 1) * 128],
                                    eye_sb[:])
                nc.vector.tensor_copy(
                    vt[:, tch, :, 0:64],
                    pst[:].rearrange("p (j k) -> p j k", j=2))

        # ---- projection phase: both batches up front ----
        for b in range(B):
            # Q: dc-outer across both slots so the PE streams while the
            # first batch's x tiles are still arriving.
            rot_q = rotp.tile([128, T], BF16, tag="rot", name=f"rotq_{b}")
            rots[(b, 'q')] = rot_q
            slots = [psB.tile([128, 1024], F32, tag="psB",
                              name=f"pq_{b}_{i}") for i in range(2)]
            for dc in range(DC):
                for qt in range(QT):
                    slot, j = slots[qt // 2], qt % 2
                    nc.tensor.matmul(
                        slot[:, j * 512:(j + 1) * 512], wq_sb[:, dc, :],
                        xts[(b, dc)][:, qt * 512:(qt + 1) * 512],
                        start=(dc == 0), stop=(dc == DC - 1))
            for half in range(2):
                rope_chain(b, slots[half], rot_q, half)

            rot_k = rotp.tile([128, T], BF16, tag="rot", name=f"rotk_{b}")
            rots[(b, 'k')] = rot_k
            for half in range(2):
                slot = psB.tile([128, 1024], F32, tag="psB",
                                name=f"pk_{b}_{half}")
                for dc in range(DC):
                    for j in range(2):
                        qt = half * 2 + j
                        nc.tensor.matmul(
                            slot[:, j * 512:(j + 1) * 512], wk_sb[:, dc, :],
                            xts[(b, dc)][:, qt * 512:(qt + 1) * 512],
                            start=(dc == 0), stop=(dc == DC - 1))
                rope_chain(b, slot, rot_k, half)

            vt = vtp.tile([128, TT, HPC, 65], BF16, tag="vt", name=f"vt_{b}")
            vts_all[b] = vt
            nc.vector.memset(vt[:, :, :, 64:65], 1.0)
            for half in range(2):
                slot = psB.tile([128, 1024], F32, tag="psB",
                                name=f"pv_{b}_{half}")
                for dc in range(DC):
                    for j in range(2):
                        qt = half * 2 + j
                        nc.tensor.matmul(
                            slot[:, j * 512:(j + 1) * 512], wv_sb[:, dc, :],
                            xts[(b, dc)][:, qt * 512:(qt + 1) * 512],
                            start=(dc == 0), stop=(dc == DC - 1))
                v_chain(b, slot, half)

        # ---- attention + wo ----
        pv_pending = deque()
        wo_pending = deque()
        wo_queue = []

        def drain_pv():
            while pv_pending:
                pv_pending.popleft()()

        def drain_wo(n):
            c = 0
            while wo_pending and c < n:
                wo_pending.popleft()()
                c += 1

        def schedule_wo(b, qt, cat, final=False):
            osb = osbp.tile([128, DC, 512], BF16, tag="osb",
                            name=f"osb_{b}_{qt}")
            for ec in range(DC):
                def emit_ec(ec=ec, b=b, qt=qt, cat=cat, osb=osb, final=final):
                    po = psO.tile([128, 512], F32, tag="psO",
                                  name=f"po_{b}_{qt}_{ec}")
                    nc.tensor.matmul(po[:], wo_sb[:, ec, :], cat[:],
                                     start=True, stop=True)
                    if final and ec % 2 == 1:
                        nc.scalar.copy(osb[:, ec, :], po[:])
                    else:
                        nc.vector.tensor_copy(osb[:, ec, :], po[:])
                    if ec == DC - 1:
                        for i in range(4):
                            nc.sync.dma_start(
                                outT[b, qt, 32 * i:32 * i + 32],
                                osb[32 * i:32 * i + 32])
                wo_pending.append(emit_ec)

        for b in range(B):
            qrot, krot = rots[(b, 'q')], rots[(b, 'k')]
            vt = vts_all[b]
            for qt in range(QT):
                nkb = 4 * qt + 4
                cat = catp.tile([128, 512], BF16, tag="cat",
                                name=f"cat_{b}_{qt}")
                if wo_queue:
                    schedule_wo(*wo_queue.pop(0))
                for h in range(HPC):
                    ph = psH.tile([65, 512], F32, tag="psH",
                                  name=f"ph_{b}_{qt}_{h}")
                    for pair in range(nkb // 2):
                        kb0, kb1 = 2 * pair, 2 * pair + 1
                        slot = psB.tile([128, 1024], F32, tag="psB",
                                        name=f"ps_{b}_{qt}_{h}_{pair}")
                        pt = ptp.tile([128, 1024], BF16, tag="pt",
                                      name=f"pt_{b}_{qt}_{h}_{pair}")
                        for j, kb in ((0, kb0), (1, kb1)):
                            c0 = max(0, kb - 4 * qt) * 128
                            nc.tensor.matmul(
                                slot[:, j * 512 + c0:(j + 1) * 512],
                                krot[64 * h:64 * h + 64,
                                     kb * 128:(kb + 1) * 128],
                                qrot[64 * h:64 * h + 64,
                                     qt * 512 + c0:(qt + 1) * 512],
                                start=True, stop=True)
                        c0a = max(0, kb0 - 4 * qt) * 128
                        nc.scalar.activation(
                            pt[:, c0a:1024], slot[:, c0a:1024], EXP,
                            scale=float(1.0 / np.sqrt(DK)))
                        for j, kb in ((0, kb0), (1, kb1)):
                            if kb >= 4 * qt:
                                c0 = (kb - 4 * qt) * 128
                                nc.gpsimd.tensor_mul(
                                    pt[:, j * 512 + c0:j * 512 + c0 + 128],
                                    pt[:, j * 512 + c0:j * 512 + c0 + 128],
                                    tri_sb[:])

                        def mk(pt=pt, ph=ph, kb0=kb0, kb1=kb1, qt=qt, h=h,
                               nkb=nkb, cat=cat, vt=vt,
                               last=(pair == nkb // 2 - 1)):
                            def go():
                                for j, kb in ((0, kb0), (1, kb1)):
                                    c0 = max(0, kb - 4 * qt) * 128
                                    nc.tensor.matmul(
                                        ph[:, c0:512], vt[:, kb, h, 0:65],
                                        pt[:, j * 512 + c0:(j + 1) * 512],
                                        start=(kb == 0), stop=(kb == nkb - 1))
                                if last:
                                    lrow = recp.tile([1, 512], F32,
                                                     tag="lrow")
                                    nc.vector.tensor_copy(lrow[:],
                                                          ph[64:65, :])
                                    rec = recp.tile([1, 512], F32, tag="rec")
                                    nc.vector.reciprocal_approx_fast(
                                        rec[:], lrow[:])
                                    pbs = recp.tile([64, 512], F32,
                                                    tag="pbs")
                                    nc.gpsimd.partition_broadcast(
                                        pbs[:], rec[:], channels=64)
                                    nc.vector.tensor_mul(
                                        cat[h * 64:(h + 1) * 64, :],
                                        ph[0:64, :], pbs[:])
                            return go

                        drain_pv()
                        pv_pending.append(mk())
                        drain_wo(2)
                    drain_wo(99)
                wo_queue.append((b, qt, cat))
            drain_pv()
        while wo_queue:
            schedule_wo(*wo_queue.pop(0), final=(len(wo_queue) == 0))
            drain_wo(99)
    nc.compile()
    return nc


_NC_CACHE = None


def _get_nc():
    global _NC_CACHE
    if _NC_CACHE is None:
        _NC_CACHE = build_nc()
    return _NC_CACHE


def make_inputs(x, wq, wk, wv, wo, core):
    """Per-core input prep (numpy). core in [0, 8)."""
    bf16 = ml_dtypes.bfloat16
    # xT [B, dc, 128, T]: partition-major so each DMA run is 4KB/partition
    xt = np.ascontiguousarray(
        x.transpose(0, 2, 1).reshape(B, DC, 128, T)).astype(bf16)

    # per-head even/odd de-interleave permutation for q/k rows
    perm64 = np.concatenate([np.arange(0, 64, 2), np.arange(1, 64, 2)])
    rows = core * 128 + (np.arange(128) // 64) * 64 + perm64[np.arange(128) % 64]
    rows_plain = core * 128 + np.arange(128)

    def wT_blocks(w, rws):
        # [128 d, dc, e] with [d, dc, e] = w[rws[e], dc*128 + d]
        return np.ascontiguousarray(
            w[rws, :].T.reshape(DC, 128, E).transpose(1, 0, 2)).astype(bf16)

    wqT = wT_blocks(wq, rows)
    wkT = wT_blocks(wk, rows)
    wvT = wT_blocks(wv, rows_plain)
    # woT [d_local, ec, e_out] = wo[ec*128 + e_out, core*128 + d_local]
    woT = np.ascontiguousarray(
        wo[:, core * 128:(core + 1) * 128].reshape(DC, 128, 128)
        .transpose(2, 0, 1)).astype(bf16)

    inv = ROPE_THETA ** (-2.0 * np.arange(DK // 2) / DK)
    ang = np.arange(T)[None, :] * inv[:, None]          # [32, T]
    cos32 = np.cos(ang)
    sin32 = np.sin(ang)
    # rot[p] = psb[p]*ct[p] + psb[p^32]*st[p^32]  (swap done on the m2 term)
    ctab = np.tile(cos32, (4, 1)).astype(bf16)
    stab = np.tile(np.concatenate([sin32, -sin32], axis=0), (2, 1)).astype(bf16)
    tri = (np.arange(128)[:, None] <= np.arange(128)[None, :]).astype(bf16)
    eye = np.eye(128).astype(bf16)

    return {
        "xT": xt, "wqT": wqT, "wkT": wkT, "wvT": wvT, "woT": woT,
        "ctab": ctab, "stab": stab, "tri": tri, "eye": eye,
    }


def gather_output(results):
    """Sum per-core partials and restore [B, T, D] layout."""
    acc = None
    for res in results:
        o = np.asarray(res["outT"], dtype=np.float32)
        acc = o if acc is None else acc + o
    # outT[b, qt, p, ec, q] -> out[b, qt*512+q, ec*128+p]
    return np.ascontiguousarray(
        acc.transpose(0, 1, 4, 3, 2).reshape(B, T, D))


def kernel(x, wq, wk, wv, wo, trace=False, **run_kwargs):
    from concourse.bass_utils import run_bass_kernel_spmd

    x = np.asarray(x, dtype=np.float32)
    wq = np.asarray(wq, dtype=np.float32)
    wk = np.asarray(wk, dtype=np.float32)
    wv = np.asarray(wv, dtype=np.float32)
    wo = np.asarray(wo, dtype=np.float32)

    nc = _get_nc()
    in_maps = [make_inputs(x, wq, wk, wv, wo, c) for c in range(NCORES)]
    res = run_bass_kernel_spmd(nc, in_maps, core_ids=list(range(NCORES)),
                               trace=trace, **run_kwargs)
    out = gather_output(res.results)
    kernel.last_results = res
    return out


# revision 22
# speedup vs baseline: 1.4407x; 1.4407x over previous
"""Causal multi-head self-attention with RoPE on 8 Trainium2 NeuronCores.

Problem shapes (hardcoded): x [2, 2048, 1024], wq/wk/wv/wo [1024, 1024],
16 heads, head dim 64, rope theta 1000.0.

Sharding: tensor-parallel over heads -- each of the 8 cores owns 2 heads
(128 of the 1024 hidden dims). wq/wk/wv are column-sharded (rows of the
[out, in] weights), wo is row-sharded; the all-reduce of the 8 partial
outputs is done on the host during the gather/unshard step.

Design notes (v2 -- tuned for warm-PE occupancy):
  - PE cost on TRN2 is 1 moving-column/cycle with a HAM clock gate
    (2.4 GHz warm / 1.2 GHz cold), so the schedule aims for a dense,
    uninterrupted matmul stream: all projections (both batches) first,
    then attention, with wo-projections deferred one query-tile and
    interleaved into the attention matmul stream.
  - RoPE pairs are permuted to (row r, row r+32) within each 64-row head
    block; rot = psb*C + swap32(psb*S') where the partition swap runs as
    4 small SBUF->SBUF DMAs off the critical engines.
  - Scores are computed as S^T tiles [128 keys, 512 queries]; two key
    blocks share one 2-bank PSUM slot so exp() runs as [128, 1024] calls
    (scalar-engine activation overhead is 352 cycles/call).
  - Softmax needs no max-subtraction (scores are O(1)); the denominator
    comes from a ones-column appended to V; normalization uses a gpsimd
    partition_broadcast of the reciprocal row.
  - All matmul inputs bf16; fp32 PSUM accumulation; bf16 partial outputs
    (summed in fp32 on host).
"""

import sys

sys.path.insert(0, "/opt/trn_rl_repo")

from collections import deque

import ml_dtypes
import numpy as np

import concourse.bacc as bacc
import concourse.tile as tile
from concourse import mybir

F32 = mybir.dt.float32
BF16 = mybir.dt.bfloat16
EXP = mybir.ActivationFunctionType.Exp

B = 2
T = 2048
D = 1024
H = 16
DK = 64
NCORES = 8
HPC = H // NCORES      # heads per core = 2
E = HPC * DK           # local out dims per core = 128
DC = D // 128          # 8 chunks of the contraction dim
QT = T // 512          # 4 query tiles of 512
TT = T // 128          # 16 key/value tiles of 128
ROPE_THETA = 1000.0


def build_nc():
    nc = bacc.Bacc("TRN2", target_bir_lowering=False, debug=False,
                   num_devices=NCORES)

    xT = nc.dram_tensor("xT", [B, DC, 128, T], BF16, kind="ExternalInput")
    wqT = nc.dram_tensor("wqT", [128, DC, E], BF16, kind="ExternalInput")
    wkT = nc.dram_tensor("wkT", [128, DC, E], BF16, kind="ExternalInput")
    wvT = nc.dram_tensor("wvT", [128, DC, E], BF16, kind="ExternalInput")
    woT = nc.dram_tensor("woT", [128, DC, 128], BF16, kind="ExternalInput")
    ctab = nc.dram_tensor("ctab", [128, T], BF16, kind="ExternalInput")
    stab = nc.dram_tensor("stab", [128, T], BF16, kind="ExternalInput")
    tri = nc.dram_tensor("tri", [128, 128], BF16, kind="ExternalInput")
    eye = nc.dram_tensor("eye", [128, 128], BF16, kind="ExternalInput")
    ones2 = nc.dram_tensor("ones2", [2, 128], BF16, kind="ExternalInput")
    outT = nc.dram_tensor("outT", [B, QT, 128, DC, 512], BF16,
                          kind="ExternalOutput")

    from contextlib import ExitStack

    with tile.TileContext(nc) as tc, ExitStack() as est:
        pool = lambda name, bufs, **kw: est.enter_context(
            tc.tile_pool(name=name, bufs=bufs, **kw))
        constp = pool("const", 1)
        xtp = pool("xt", 2 * DC)
        rotp = pool("rot", 4)
        psbp = pool("psb", 2)
        m1p = pool("m1", 2)
        m2p = pool("m2", 2)
        m2sp = pool("m2s", 2)
        vtsp = pool("vts", 2)
        vtp = pool("vt", 2)
        ptp = pool("pt", 3)
        catp = pool("cat", 3)
        osbp = pool("osb", 2)
        recp = pool("rec", 2)
        psB = pool("psB", 2, space="PSUM")   # proj accum + S^T pair slots
        psH = pool("psH", 2, space="PSUM")   # head accum [65, 512]
        psO = pool("psO", 2, space="PSUM")   # wo out + V transposes

        # ---- constant tiles ----
        wq_sb = constp.tile([128, DC, E], BF16, tag="wq")
        wk_sb = constp.tile([128, DC, E], BF16, tag="wk")
        wv_sb = constp.tile([128, DC, E], BF16, tag="wv")
        wo_sb = constp.tile([128, DC, 128], BF16, tag="wo")
        ct_sb = constp.tile([128, T], BF16, tag="ct")
        st_sb = constp.tile([128, T], BF16, tag="st")
        tri_sb = constp.tile([128, 128], BF16, tag="tri")
        eye_sb = constp.tile([128, 128], BF16, tag="eye")
        # ones2[h, m] = 1 iff head h owns output row m -> the broadcast
        # matmul ones2.T @ recb2 replicates each head's reciprocal row
        # into its 64 output partitions in a single PE op.
        ones_sb = constp.tile([2, 128], BF16, tag="ones")

        # Warm the scalar-engine exp tables while the input DMAs run.
        dum = constp.tile([128, 16], F32, tag="dum")
        nc.vector.memset(dum[:], 0.0)
        nc.scalar.activation(dum[:], dum[:], EXP)

        xts = {}
        for b in range(B):
            for dc in range(DC):
                xts[(b, dc)] = xtp.tile([128, T], BF16, tag="xt",
                                        name=f"xt_{b}_{dc}")

        # Input DMA triggers issue serially (~0.6us each) per engine queue,
        # so spread them across four otherwise-idle queues and put the
        # first-needed tensors first on each.
        nc.sync.dma_start(wq_sb[:], wqT[:])
        for dc in range(DC):
            nc.sync.dma_start(xts[(0, dc)][:], xT[0, dc])
        nc.scalar.dma_start(ct_sb[:], ctab[:])
        nc.scalar.dma_start(st_sb[:], stab[:])
        nc.scalar.dma_start(wk_sb[:], wkT[:])
        nc.scalar.dma_start(wv_sb[:], wvT[:])
        nc.scalar.dma_start(wo_sb[:], woT[:])
        nc.gpsimd.dma_start(tri_sb[:], tri[:])
        nc.gpsimd.dma_start(eye_sb[:], eye[:])
        nc.gpsimd.dma_start(ones_sb[:], ones2[:])
        for dc in range(DC):
            nc.gpsimd.dma_start(xts[(1, dc)][:], xT[1, dc])

        rots = {}   # (b, 'q'|'k') -> [128, T] bf16
        vts_all = {}  # b -> vt tile [128, TT, HPC, 65]

        def rope_chain(b, slot, rot, half):
            co = half * 1024
            psb = psbp.tile([128, 1024], BF16, tag="psb")
            nc.scalar.copy(psb[:], slot[:])
            m1 = m1p.tile([128, 1024], BF16, tag="m1")
            nc.vector.tensor_mul(m1[:], psb[:], ct_sb[:, co:co + 1024])
            m2 = m2p.tile([128, 1024], BF16, tag="m2")
            nc.gpsimd.tensor_mul(m2[:], psb[:], st_sb[:, co:co + 1024])
            m2s = m2sp.tile([128, 1024], BF16, tag="m2s")
            for blk in range(4):
                s = (blk ^ 1) * 32
                nc.sync.dma_start(m2s[blk * 32:blk * 32 + 32, :],
                                  m2[s:s + 32, :])
            nc.gpsimd.tensor_add(rot[:, co:co + 1024], m1[:], m2s[:])

        def v_chain(b, slot, half):
            vt = vts_all[b]
            vts = vtsp.tile([128, 1024], BF16, tag="vts")
            nc.scalar.copy(vts[:], slot[:])
            for j4 in range(8):
                tch = half * 8 + j4
                pst = psO.tile([128, 128], BF16, tag="psO",
                               name=f"pst_{b}_{tch}")
                nc.tensor.transpose(pst[:], vts[:, j4 * 128:(j4 + 1) * 128],
                                    eye_sb[:])
                nc.vector.tensor_copy(
                    vt[:, tch, :, 0:64],
                    pst[:].rearrange("p (j k) -> p j k", j=2))

        # ---- projection phase: both batches up front ----
        for b in range(B):
            # Q: dc-outer across both slots so the PE streams while the
            # first batch's x tiles are still arriving.
            rot_q = rotp.tile([128, T], BF16, tag="rot", name=f"rotq_{b}")
            rots[(b, 'q')] = rot_q
            slots = [psB.tile([128, 1024], F32, tag="psB",
                              name=f"pq_{b}_{i}") for i in range(2)]
            for dc in range(DC):
                for qt in range(QT):
                    slot, j = slots[qt // 2], qt % 2
                    nc.tensor.matmul(
                        slot[:, j * 512:(j + 1) * 512], wq_sb[:, dc, :],
                        xts[(b, dc)][:, qt * 512:(qt + 1) * 512],
                        start=(dc == 0), stop=(dc == DC - 1))
            for half in range(2):
                rope_chain(b, slots[half], rot_q, half)

            rot_k = rotp.tile([128, T], BF16, tag="rot", name=f"rotk_{b}")
            rots[(b, 'k')] = rot_k
            for half in range(2):
                slot = psB.tile([128, 1024], F32, tag="psB",
                                name=f"pk_{b}_{half}")
                for dc in range(DC):
                    for j in range(2):
                        qt = half * 2 + j
                        nc.tensor.matmul(
                            slot[:, j * 512:(j + 1) * 512], wk_sb[:, dc, :],
                            xts[(b, dc)][:, qt * 512:(qt + 1) * 512],
                            start=(dc == 0), stop=(dc == DC - 1))
                rope_chain(b, slot, rot_k, half)

            vt = vtp.tile([128, TT, HPC, 65], BF16, tag="vt", name=f"vt_{b}")
            vts_all[b] = vt
            nc.vector.memset(vt[:, :, :, 64:65], 1.0)
            for half in range(2):
                slot = psB.tile([128, 1024], F32, tag="psB",
                                name=f"pv_{b}_{half}")
                for dc in range(DC):
                    for j in range(2):
                        qt = half * 2 + j
                        nc.tensor.matmul(
                            slot[:, j * 512:(j + 1) * 512], wv_sb[:, dc, :],
                            xts[(b, dc)][:, qt * 512:(qt + 1) * 512],
                            start=(dc == 0), stop=(dc == DC - 1))
                v_chain(b, slot, half)

        # ---- attention + wo ----
        pv_pending = deque()
        wo_pending = deque()
        wo_queue = []

        def drain_pv():
            n = len(pv_pending)
            for _ in range(n):
                pv_pending.popleft()()

        def drain_wo(n):
            c = 0
            while wo_pending and c < n:
                wo_pending.popleft()()
                c += 1

        def schedule_wo(b, qt, cat, split=False):
            osb = osbp.tile([128, DC, 512], BF16, tag="osb",
                            name=f"osb_{b}_{qt}")
            for ec in range(DC):
                def emit_ec(ec=ec, b=b, qt=qt, cat=cat, osb=osb, split=split):
                    po = psO.tile([128, 512], F32, tag="psO",
                                  name=f"po_{b}_{qt}_{ec}")
                    nc.tensor.matmul(po[:], wo_sb[:, ec, :], cat[:],
                                     start=True, stop=True)
                    if split and ec % 2 == 1:
                        nc.scalar.copy(osb[:, ec, :], po[:])
                    else:
                        nc.vector.tensor_copy(osb[:, ec, :], po[:])
                    if ec == DC - 1:
                        nch = 8 if split else 4
                        step = 128 // nch
                        for i in range(nch):
                            nc.sync.dma_start(
                                outT[b, qt, step * i:step * (i + 1)],
                                osb[step * i:step * (i + 1)])
                wo_pending.append(emit_ec)

        for b in range(B):
            qrot, krot = rots[(b, 'q')], rots[(b, 'k')]
            vt = vts_all[b]
            for qt in range(QT):
                nkb = 4 * qt + 4
                cat = catp.tile([128, 512], BF16, tag="cat",
                                name=f"cat_{b}_{qt}")
                lrow = recp.tile([2, 512], F32, tag="lrow",
                                 name=f"lrow_{b}_{qt}")
                phs = {}
                if wo_queue:
                    schedule_wo(*wo_queue.pop(0))
                for h in range(HPC):
                    ph = psH.tile([65, 512], F32, tag="psH",
                                  name=f"ph_{b}_{qt}_{h}")
                    phs[h] = ph
                    for pair in range(nkb // 2):
                        kb0, kb1 = 2 * pair, 2 * pair + 1
                        slot = psB.tile([128, 1024], F32, tag="psB",
                                        name=f"ps_{b}_{qt}_{h}_{pair}")
                        pt = ptp.tile([128, 1024], BF16, tag="pt",
                                      name=f"pt_{b}_{qt}_{h}_{pair}")
                        for j, kb in ((0, kb0), (1, kb1)):
                            c0 = max(0, kb - 4 * qt) * 128
                            nc.tensor.matmul(
                                slot[:, j * 512 + c0:(j + 1) * 512],
                                krot[64 * h:64 * h + 64,
                                     kb * 128:(kb + 1) * 128],
                                qrot[64 * h:64 * h + 64,
                                     qt * 512 + c0:(qt + 1) * 512],
                                start=True, stop=True)
                        c0a = max(0, kb0 - 4 * qt) * 128
                        nc.scalar.activation(
                            pt[:, c0a:1024], slot[:, c0a:1024], EXP,
                            scale=float(1.0 / np.sqrt(DK)))
                        for j, kb in ((0, kb0), (1, kb1)):
                            if kb >= 4 * qt:
                                c0 = (kb - 4 * qt) * 128
                                nc.gpsimd.tensor_mul(
                                    pt[:, j * 512 + c0:j * 512 + c0 + 128],
                                    pt[:, j * 512 + c0:j * 512 + c0 + 128],
                                    tri_sb[:])

                        def mk(pt=pt, ph=ph, kb0=kb0, kb1=kb1, qt=qt, h=h,
                               nkb=nkb, cat=cat, vt=vt, lrow=lrow, phs=phs,
                               last=(pair == nkb // 2 - 1)):
                            def go():
                                for j, kb in ((0, kb0), (1, kb1)):
                                    c0 = max(0, kb - 4 * qt) * 128
                                    nc.tensor.matmul(
                                        ph[:, c0:512], vt[:, kb, h, 0:65],
                                        pt[:, j * 512 + c0:(j + 1) * 512],
                                        start=(kb == 0), stop=(kb == nkb - 1))
                                if not last:
                                    return
                                # phase A: stage this head's ones-row sum.
                                # (engine writes need 32-aligned partition
                                # bases, so head 1 bounces through SBUF and
                                # a 1-descriptor DMA into partition 1.)
                                if h == 0:
                                    nc.vector.tensor_copy(lrow[0:1, :],
                                                          ph[64:65, :])
                                    return
                                l1 = recp.tile([1, 512], F32, tag="l1")
                                nc.vector.tensor_copy(l1[:], ph[64:65, :])
                                nc.sync.dma_start(lrow[1:2, :], l1[:])
                                rec = recp.tile([2, 512], F32, tag="rec")
                                nc.vector.reciprocal_approx_fast(
                                    rec[:], lrow[:])
                                recb = recp.tile([2, 512], BF16, tag="recb")
                                nc.vector.tensor_copy(recb[:], rec[:])

                                def norm_b(cat=cat, phs=phs, recb=recb):
                                    # phase B (one pair later): the PE
                                    # broadcast matmul's operand is ready
                                    # by now, so it never stalls the
                                    # in-order PE queue.
                                    pb = psO.tile([128, 512], F32,
                                                  tag="psO")
                                    nc.tensor.matmul(
                                        pb[:], ones_sb[:], recb[:],
                                        start=True, stop=True)
                                    pbs = recp.tile([128, 512], BF16,
                                                    tag="pbs")
                                    nc.vector.tensor_copy(pbs[:], pb[:])
                                    nc.vector.tensor_mul(
                                        cat[0:64, :], phs[0][0:64, :],
                                        pbs[0:64, :])
                                    nc.vector.tensor_mul(
                                        cat[64:128, :], phs[1][0:64, :],
                                        pbs[64:128, :])
                                pv_pending.append(norm_b)
                            return go

                        drain_pv()
                        pv_pending.append(mk())
                        if pair > 0:
                            drain_wo(2)
                wo_queue.append((b, qt, cat))
            drain_pv()
            drain_pv()
        while wo_queue:
            schedule_wo(*wo_queue.pop(0), split=(len(wo_queue) == 0))
            drain_wo(99)
        drain_pv()
    nc.compile()
    return nc


_NC_CACHE = None


def _get_nc():
    global _NC_CACHE
    if _NC_CACHE is None:
        _NC_CACHE = build_nc()
    return _NC_CACHE


def make_inputs(x, wq, wk, wv, wo, core):
    """Per-core input prep (numpy). core in [0, 8)."""
    bf16 = ml_dtypes.bfloat16
    # xT [B, dc, 128, T]: partition-major so each DMA run is 4KB/partition
    xt = np.ascontiguousarray(
        x.transpose(0, 2, 1).reshape(B, DC, 128, T)).astype(bf16)

    # per-head even/odd de-interleave permutation for q/k rows
    perm64 = np.concatenate([np.arange(0, 64, 2), np.arange(1, 64, 2)])
    rows = core * 128 + (np.arange(128) // 64) * 64 + perm64[np.arange(128) % 64]
    rows_plain = core * 128 + np.arange(128)

    def wT_blocks(w, rws):
        # [128 d, dc, e] with [d, dc, e] = w[rws[e], dc*128 + d]
        return np.ascontiguousarray(
            w[rws, :].T.reshape(DC, 128, E).transpose(1, 0, 2)).astype(bf16)

    wqT = wT_blocks(wq, rows)
    wkT = wT_blocks(wk, rows)
    wvT = wT_blocks(wv, rows_plain)
    # woT [d_local, ec, e_out] = wo[ec*128 + e_out, core*128 + d_local]
    woT = np.ascontiguousarray(
        wo[:, core * 128:(core + 1) * 128].reshape(DC, 128, 128)
        .transpose(2, 0, 1)).astype(bf16)

    inv = ROPE_THETA ** (-2.0 * np.arange(DK // 2) / DK)
    ang = np.arange(T)[None, :] * inv[:, None]          # [32, T]
    cos32 = np.cos(ang)
    sin32 = np.sin(ang)
    # rot[p] = psb[p]*ct[p] + psb[p^32]*st[p^32]  (swap done on the m2 term)
    ctab = np.tile(cos32, (4, 1)).astype(bf16)
    stab = np.tile(np.concatenate([sin32, -sin32], axis=0), (2, 1)).astype(bf16)
    tri = (np.arange(128)[:, None] <= np.arange(128)[None, :]).astype(bf16)
    eye = np.eye(128).astype(bf16)
    ones2 = (np.arange(128)[None, :] // 64 == np.arange(2)[:, None]).astype(bf16)

    return {
        "xT": xt, "wqT": wqT, "wkT": wkT, "wvT": wvT, "woT": woT,
        "ctab": ctab, "stab": stab, "tri": tri, "eye": eye, "ones2": ones2,
    }


def gather_output(results):
    """Sum per-core partials and restore [B, T, D] layout."""
    acc = None
    for res in results:
        o = np.asarray(res["outT"], dtype=np.float32)
        acc = o if acc is None else acc + o
    # outT[b, qt, p, ec, q] -> out[b, qt*512+q, ec*128+p]
    return np.ascontiguousarray(
        acc.transpose(0, 1, 4, 3, 2).reshape(B, T, D))


def kernel(x, wq, wk, wv, wo, trace=False, **run_kwargs):
    from concourse.bass_utils import run_bass_kernel_spmd

    x = np.asarray(x, dtype=np.float32)
    wq = np.asarray(wq, dtype=np.float32)
    wk = np.asarray(wk, dtype=np.float32)
    wv = np.asarray(wv, dtype=np.float32)
    wo = np.asarray(wo, dtype=np.float32)

    nc = _get_nc()
    in_maps = [make_inputs(x, wq, wk, wv, wo, c) for c in range(NCORES)]
    res = run_bass_kernel_spmd(nc, in_maps, core_ids=list(range(NCORES)),
                               trace=trace, **run_kwargs)
    out = gather_output(res.results)
    kernel.last_results = res
    return out
